# revision 4
# baseline (speedup 1.0000x reference)
# Trainium2 Bass kernel for nn_LNKillingRelu: out = where(kf<=0, x, x + kf*d)
#   d  = einsum('fkn,gf->gkn', x, W)                      (per batch)
#   kf = einsum('fkn,kl,fln->fn', x, G, d)  broadcast over k
# G (Killing Gram of sl(3)): G[0,0]=G[4,4]=12, G[0,4]=G[4,0]=-6,
#   G[1,3]=G[3,1]=G[2,6]=G[6,2]=G[5,7]=G[7,5]=6; with kf' = kf/6:
#   kf' = x0*(2d0-d4) + x4*(2d4-d0) + x1*d3 + x3*d1 + x2*d6 + x6*d2
#       + x5*d7 + x7*d5
#   out = x + relu(6*kf') * d
#
# v7 (v6 + DMA diet; v6 trace showed DVE 8.3us/iter but DMA active 172us
# and early-phase DVE stalls from Sync trigger congestion):
#  - host rearranges x/xaux/out to chunk-contiguous [nch, F, ..., nt] so
#    each x chunk is ONE 4MB DMA with 8KB-contiguous rows (3-dim APs),
#    aux is 4 upfront DMAs into one resident tile, out stores are fully
#    contiguous.  40 in-triggers -> 7.
#  - t1 of the kf reduction is split: p[4:6]+=p[0:2] rides the SWDGE
#    accum-DMA (CCE add), p[6:8]+=p[2:4] stays on DVE; balances the
#    SBUF fabric (3x payload for RMW) against DVE time.
#  - rest of v6 kept: 1-wide gate + broadcast og, flat single-op og/o2,
#    reordered 3-op products, no pstate warmup.
#
# Sharding: data-parallel over batch B=8 -> one batch per NeuronCore.

import os
from contextlib import ExitStack

import numpy as np

import concourse.bass as bass
import concourse.mybir as mybir
import concourse.tile as tile
from concourse.bass_utils import run_bass_kernel_spmd

B, F, K, N = 8, 512, 8, 2048
P = 128
FT = F // P  # 4 channel tiles
KH = K // 2  # planes per PSUM half

f32 = mybir.dt.float32
f16 = mybir.dt.float16
Alu = mybir.AluOpType
ActF = mybir.ActivationFunctionType

# t1 mode: 0 = all DVE, 1 = all SWDGE, 2 = half/half (default)
T1_MODE = int(os.environ.get("V7_T1", "2"))


def _ap(base, off_elems, dims):
    """Raw AP from a base AP: keep partition dim, replace free dims."""
    return bass.AP(
        tensor=base.tensor,
        offset=base.offset + off_elems,
        ap=[base.ap[0]] + dims,
    )


def _rap(base, off_elems, dims):
    """Fully raw AP (partition dim included in dims)."""
    return bass.AP(tensor=base.tensor, offset=off_elems, ap=dims)


def build_nc(n_total=N, nt=512):
    nch = n_total // nt
    KN = K * nt          # elems per (f row, chunk) = 4096
    nc = bass.Bass(detect_race_conditions=False)
    # chunk-contiguous layouts, host-rearranged
    xr = nc.dram_tensor("xr", [nch, F, K, nt], f16, kind="ExternalInput")
    wt = nc.dram_tensor("wt", [F, F], f16, kind="ExternalInput")  # W^T (f, g)
    auxr = nc.dram_tensor("auxr", [nch, F, 2, nt], f16, kind="ExternalInput")
    outr = nc.dram_tensor("outr", [nch, F, K, nt], f16, kind="ExternalOutput")
    xr_b = xr[:, :, :, :]
    auxr_b = auxr[:, :, :, :]
    outr_b = outr[:, :, :, :]

    with tile.TileContext(nc) as tc, ExitStack() as ctx:
        wpool = ctx.enter_context(tc.tile_pool(name="w", bufs=1))
        xpool = ctx.enter_context(tc.tile_pool(name="xc", bufs=3))
        papool = ctx.enter_context(tc.tile_pool(name="pda", bufs=1, space="PSUM"))
        pbpool = ctx.enter_context(tc.tile_pool(name="pdb", bufs=1, space="PSUM"))
        dcpool = ctx.enter_context(tc.tile_pool(name="dc", bufs=3))
        prpool = ctx.enter_context(tc.tile_pool(name="prod", bufs=2))
        s2pool = ctx.enter_context(tc.tile_pool(name="s2", bufs=2))
        s3pool = ctx.enter_context(tc.tile_pool(name="s3", bufs=3))
        opool = ctx.enter_context(tc.tile_pool(name="og", bufs=2))

        # resident W^T tiles: wsb[ft][p, g] , f = ft*128+p
        wsb = []
        for ft in range(FT):
            w_t = wpool.tile([P, F], f16, tag=f"w{ft}")
            nc.sync.dma_start(out=w_t[:], in_=wt[ft * P : (ft + 1) * P, :])
            wsb.append(w_t)

        # resident aux tile: [P, nch, FT, 2, nt] flattened
        auxt = wpool.tile([P, nch * FT * 2 * nt], f16, tag="aux")
        for c in range(nch):
            nc.sync.dma_start(
                out=_ap(auxt[:], c * FT * 2 * nt, [[2 * nt, FT], [1, 2 * nt]]),
                in_=_rap(
                    auxr_b,
                    c * F * 2 * nt,
                    [[2 * nt, P], [P * 2 * nt, FT], [1, 2 * nt]],
                ),
            )

        # Walrus only allows ONE sync wait per Matmult (waits ride the
        # LDWEIGHTS struct).  Warmup matmuls make PE observe each W-DMA
        # semaphore individually so later matmuls never wait on W.
        warm = papool.tile([P, KH, nt], f32, tag="pda")
        for ft in range(FT):
            nc.tensor.matmul(
                warm[:, 0, 0:1], wsb[ft][:, 0:P], wsb[ft][:, 0:1], start=True, stop=True
            )

        def emit_products(st):
            # p slots: 0..3 <- x(3,1,7,5)*d(1,3,5,7); 4,5 <- x(6,2)*d(2,6);
            #          6,7 <- aux(0,1)*d(0,4)
            xt, xo, dc = st["xt"], st["xo"], st["dc"]
            p = prpool.tile([P, K, nt], f16, tag="p")
            nc.vector.tensor_tensor(
                out=_ap(p[:], 0, [[1, 4 * nt]]),
                in0=_ap(xt, xo + 3 * nt, [[4 * nt, 2], [-2 * nt, 2], [1, nt]]),
                in1=_ap(dc[:], nt, [[4 * nt, 2], [2 * nt, 2], [1, nt]]),
                op=Alu.mult,
            )
            nc.vector.tensor_tensor(
                out=_ap(p[:], 4 * nt, [[1, 2 * nt]]),
                in0=_ap(xt, xo + 6 * nt, [[-4 * nt, 2], [1, nt]]),
                in1=_ap(dc[:], 2 * nt, [[4 * nt, 2], [1, nt]]),
                op=Alu.mult,
            )
            nc.vector.tensor_tensor(
                out=_ap(p[:], 6 * nt, [[1, 2 * nt]]),
                in0=st["ax"],
                in1=_ap(dc[:], 0, [[4 * nt, 2], [1, nt]]),
                op=Alu.mult,
            )
            st["p"] = p

        def emit_t1(st):
            p = st["p"]
            if T1_MODE == 1:
                nc.gpsimd.dma_start(
                    out=p[:, 4:8, :], in_=p[:, 0:4, :], accum_op=Alu.add
                )
            elif T1_MODE == 2:
                nc.gpsimd.dma_start(
                    out=p[:, 4:6, :], in_=p[:, 0:2, :], accum_op=Alu.add
                )
                nc.vector.tensor_tensor(
                    out=_ap(p[:], 6 * nt, [[1, 2 * nt]]),
                    in0=_ap(p[:], 2 * nt, [[1, 2 * nt]]),
                    in1=_ap(p[:], 6 * nt, [[1, 2 * nt]]),
                    op=Alu.add,
                )
            else:
                nc.vector.tensor_tensor(
                    out=_ap(p[:], 4 * nt, [[1, 4 * nt]]),
                    in0=_ap(p[:], 0, [[1, 4 * nt]]),
                    in1=_ap(p[:], 4 * nt, [[1, 4 * nt]]),
                    op=Alu.add,
                )

        def emit_t2_kf(st):
            p = st["p"]
            t2 = s2pool.tile([P, 2, nt], f16, tag="t2")
            nc.vector.tensor_tensor(
                out=_ap(t2[:], 0, [[1, 2 * nt]]),
                in0=_ap(p[:], 4 * nt, [[1, 2 * nt]]),
                in1=_ap(p[:], 6 * nt, [[1, 2 * nt]]),
                op=Alu.add,
            )
            kf = s3pool.tile([P, nt], f16, tag="kf")
            nc.vector.tensor_tensor(
                out=kf[:], in0=t2[:, 0, :], in1=t2[:, 1, :], op=Alu.add
            )
            st["kf"] = kf

        def emit_gate(st):
            gate = s3pool.tile([P, nt], f16, tag="gate")
            nc.scalar.activation(
                out=gate[:], in_=st["kf"][:], func=ActF.Relu, scale=6.0
            )
            st["gate"] = gate

        def emit_og_o2(st):
            dc, xt, xo = st["dc"], st["xt"], st["xo"]
            og = opool.tile([P, K, nt], f16, tag="og", bufs=1)
            nc.vector.tensor_tensor(
                out=_ap(og[:], 0, [[1, K * nt]]),
                in0=_ap(dc[:], 0, [[1, K * nt]]),
                in1=_ap(st["gate"][:], 0, [[0, K], [1, nt]]),
                op=Alu.mult,
            )
            o2 = opool.tile([P, K, nt], f16, tag="o2")
            nc.vector.tensor_tensor(
                out=_ap(o2[:], 0, [[1, K * nt]]),
                in0=_ap(og[:], 0, [[1, K * nt]]),
                in1=_ap(xt, xo, [[1, K * nt]]),
                op=Alu.add,
            )
            st["o2"] = o2

        def emit_out(st):
            c, gt = st["c"], st["gt"]
            nc.scalar.dma_start(
                out=_rap(
                    outr_b,
                    (c * F + gt * P) * KN,
                    [[KN, P], [1, KN]],
                ),
                in_=st["o2"][:],
            )

        prev = None   # iter m-1: products/t1/t2/kf/gate pending
        prev2 = None  # iter m-2: og/o2/out pending

        for c in range(nch):
            # one x tile per chunk: [P, FT*K*nt] (f = ft*128 + p)
            xt = xpool.tile([P, FT * KN], f16, tag="xc",
                            name=f"x{c}" if c == 0 else None)
            if c == 0:
                # half A (k 0:3) of all ft first: matmul half A of iter 0
                # needs only these
                for h in (0, 1):
                    nc.sync.dma_start(
                        out=_ap(xt[:], h * KH * nt, [[KN, FT], [1, KH * nt]]),
                        in_=_rap(
                            xr_b,
                            c * F * KN + h * KH * nt,
                            [[KN, P], [P * KN, FT], [1, KH * nt]],
                        ),
                    )
            else:
                nc.sync.dma_start(
                    out=xt[:],
                    in_=_rap(
                        xr_b,
                        c * F * KN,
                        [[KN, P], [P * KN, FT], [1, KN]],
                    ),
                )
            for gt in range(FT):
                xo = gt * KN
                dc = dcpool.tile([P, K, nt], f16, tag="dc")

                # ScalarE head: dcB of the PREVIOUS iter
                if prev is not None:
                    nc.scalar.copy(out=prev["dc"][:, KH:K, :], in_=prev["pdB"][:])
                # ---- matmul halves -> PSUM ----
                pds = []
                for half, pool in ((0, papool), (1, pbpool)):
                    pd = pool.tile([P, KH, nt], f32, tag=("pda", "pdb")[half])
                    # Dummy matmul absorbs the PSUM-slot-release wait
                    # (1-wait limit on Matmult structs).
                    nc.tensor.matmul(
                        pd[:, 0, 0:1], wsb[0][:, 0:P], wsb[0][:, 0:1],
                        start=True, stop=True,
                    )
                    k0 = half * KH
                    for ft in range(FT):
                        for jj in range(KH):
                            nc.tensor.matmul(
                                pd[:, jj, :],
                                wsb[ft][:, gt * P : (gt + 1) * P],
                                _ap(xt[:], ft * KN + (k0 + jj) * nt, [[1, nt]]),
                                start=(ft == 0),
                                stop=(ft == FT - 1),
                            )
                    pds.append(pd)
                    if half == 0:
                        nc.scalar.copy(out=dc[:, 0:KH, :], in_=pd[:])

                # ---- elementwise streams ----
                if prev is not None:
                    emit_products(prev)
                    emit_t1(prev)
                if prev2 is not None:
                    emit_og_o2(prev2)
                if prev is not None:
                    emit_t2_kf(prev)
                    emit_gate(prev)
                if prev2 is not None:
                    emit_out(prev2)

                prev2 = prev
                prev = {
                    "dc": dc, "xt": xt[:], "xo": xo,
                    "ax": _ap(auxt[:], (c * FT + gt) * 2 * nt, [[1, 2 * nt]]),
                    "pdB": pds[1], "c": c, "gt": gt,
                }

        # ---- drain ----
        nc.scalar.copy(out=prev["dc"][:, KH:K, :], in_=prev["pdB"][:])
        emit_products(prev)
        emit_t1(prev)
        emit_og_o2(prev2)
        emit_t2_kf(prev)
        emit_gate(prev)
        emit_out(prev2)
        emit_og_o2(prev)
        emit_out(prev)

    _split_waits(nc)
    return nc


# Engine datapath structs (Matmult/TT/STT/Act/...) only carry ONE sync wait on
# TRN2 walrus; sequencer instructions (NoOp) can each carry one more.  Hoist
# surplus waits onto same-engine NoOps placed just before the instruction.
def _split_waits(nc):
    nnop = 0
    for fn in nc.m.functions:
        for blk in fn.blocks:
            out = []
            for inst in blk.instructions:
                si = inst.sync_info
                if si is not None and si.on_wait and len(si.on_wait) > 1:
                    for w in si.on_wait[:-1]:
                        nop = mybir.InstNoOp(
                            name=f"{inst.name}-sw{nnop}",
                            opcode="NoOp",
                            engine=inst.engine,
                            sync_info=mybir.SyncInfo(on_wait=[w], on_update=[]),
                        )
                        nnop += 1
                        out.append(nop)
                    inst.sync_info = mybir.SyncInfo(
                        on_wait=[si.on_wait[-1]], on_update=list(si.on_update)
                    )
                out.append(inst)
            blk.instructions[:] = out
    return nc


_NC_CACHE = {}

NCH, NT = N // 512, 512


def _get_nc(n_total=N, nt=NT):
    key = (n_total, nt)
    if key not in _NC_CACHE:
        _NC_CACHE[key] = build_nc(n_total, nt)
    return _NC_CACHE[key]


def _to_f16(a: np.ndarray) -> np.ndarray:
    return np.ascontiguousarray(a.astype(np.float16))


def _chunked(a: np.ndarray) -> np.ndarray:
    # [F, C, N] -> chunk-contiguous [nch, F, C, nt]
    Fd, Cd, Nd = a.shape
    return np.ascontiguousarray(
        a.reshape(Fd, Cd, NCH, NT).transpose(2, 0, 1, 3)
    )


def make_in_maps(x: np.ndarray, W: np.ndarray):
    wt = _to_f16(W.T.copy())
    x16 = _to_f16(x)
    xa = np.stack(
        [2.0 * x[:, :, 0, :] - x[:, :, 4, :], 2.0 * x[:, :, 4, :] - x[:, :, 0, :]],
        axis=2,
    )
    xa16 = _to_f16(xa)
    return [
        {"xr": _chunked(x16[b]), "wt": wt, "auxr": _chunked(xa16[b])}
        for b in range(B)
    ]


def post(res) -> np.ndarray:
    # gather + un-chunk: [nch, F, K, nt] -> [F, K, N]
    return np.stack(
        [
            res.results[b]["outr"]
            .transpose(1, 2, 0, 3)
            .reshape(F, K, N)
            .astype(np.float32)
            for b in range(B)
        ],
        axis=0,
    )


def kernel(x: np.ndarray, W: np.ndarray) -> np.ndarray:
    assert x.shape == (B, F, K, N) and W.shape == (F, F)
    in_maps = make_in_maps(x, W)
    nc = _get_nc()
    res = run_bass_kernel_spmd(nc, in_maps, list(range(B)))
    return post(res)


if __name__ == "__main__":
    xs = np.random.randn(B, F, K, N).astype(np.float32)
    Ws = (np.random.randn(F, F) / np.sqrt(F)).astype(np.float32)
    o = kernel(xs, Ws)
    print(o.shape, o.dtype)


# revision 7
# speedup vs baseline: 1.1638x; 1.1638x over previous
# Trainium2 Bass kernel for nn_LNKillingRelu: out = where(kf<=0, x, x + kf*d)
#   d  = einsum('fkn,gf->gkn', x, W)                      (per batch)
#   kf = einsum('fkn,kl,fln->fn', x, G, d)  broadcast over k
# G (Killing Gram of sl(3)): G[0,0]=G[4,4]=12, G[0,4]=G[4,0]=-6,
#   G[1,3]=G[3,1]=G[2,6]=G[6,2]=G[5,7]=G[7,5]=6; with kf' = kf/6:
#   kf' = x0*(2d0-d4) + x4*(2d4-d0) + x1*d3 + x3*d1 + x2*d6 + x6*d2
#       + x5*d7 + x7*d5
#   out = x + relu(6*kf') * d
#
# v7 (v6 + DMA diet; v6 trace showed DVE 8.3us/iter but DMA active 172us
# and early-phase DVE stalls from Sync trigger congestion):
#  - host rearranges x/xaux/out to chunk-contiguous [nch, F, ..., nt] so
#    each x chunk is ONE 4MB DMA with 8KB-contiguous rows (3-dim APs),
#    aux is 4 upfront DMAs into one resident tile, out stores are fully
#    contiguous.  40 in-triggers -> 7.
#  - t1 of the kf reduction is split: p[4:6]+=p[0:2] rides the SWDGE
#    accum-DMA (CCE add), p[6:8]+=p[2:4] stays on DVE; balances the
#    SBUF fabric (3x payload for RMW) against DVE time.
#  - rest of v6 kept: 1-wide gate + broadcast og, flat single-op og/o2,
#    reordered 3-op products, no pstate warmup.
#
# Sharding: data-parallel over batch B=8 -> one batch per NeuronCore.

import os
from contextlib import ExitStack

import numpy as np

import concourse.bass as bass
import concourse.mybir as mybir
import concourse.tile as tile
from concourse.bass_utils import run_bass_kernel_spmd

B, F, K, N = 8, 512, 8, 2048
P = 128
FT = F // P  # 4 channel tiles
KH = K // 2  # planes per PSUM half

f32 = mybir.dt.float32
f16 = mybir.dt.float16
Alu = mybir.AluOpType
ActF = mybir.ActivationFunctionType

# t1 mode: 0 = all DVE, 1 = all SWDGE (default), 2 = half/half
T1_MODE = int(os.environ.get("V7_T1", "1"))


def _ap(base, off_elems, dims):
    """Raw AP from a base AP: keep partition dim, replace free dims."""
    return bass.AP(
        tensor=base.tensor,
        offset=base.offset + off_elems,
        ap=[base.ap[0]] + dims,
    )


def _rap(base, off_elems, dims):
    """Fully raw AP (partition dim included in dims)."""
    return bass.AP(tensor=base.tensor, offset=off_elems, ap=dims)


def build_nc(n_total=N, nt=512):
    nch = n_total // nt
    KN = K * nt          # elems per (f row, chunk) = 4096
    nc = bass.Bass(detect_race_conditions=False)
    # chunk-contiguous layouts, host-rearranged
    xr = nc.dram_tensor("xr", [nch, F, K, nt], f16, kind="ExternalInput")
    wt = nc.dram_tensor("wt", [F, F], f16, kind="ExternalInput")  # W^T (f, g)
    auxr = nc.dram_tensor("auxr", [nch, F, 2, nt], f16, kind="ExternalInput")
    outr = nc.dram_tensor("outr", [nch, F, K, nt], f16, kind="ExternalOutput")
    xr_b = xr[:, :, :, :]
    auxr_b = auxr[:, :, :, :]
    outr_b = outr[:, :, :, :]

    with tile.TileContext(nc) as tc, ExitStack() as ctx:
        wpool = ctx.enter_context(tc.tile_pool(name="w", bufs=1))
        xpool = ctx.enter_context(tc.tile_pool(name="xc", bufs=3))
        papool = ctx.enter_context(tc.tile_pool(name="pda", bufs=1, space="PSUM"))
        pbpool = ctx.enter_context(tc.tile_pool(name="pdb", bufs=1, space="PSUM"))
        dcpool = ctx.enter_context(tc.tile_pool(name="dc", bufs=3))
        prpool = ctx.enter_context(tc.tile_pool(name="prod", bufs=2))
        s2pool = ctx.enter_context(tc.tile_pool(name="s2", bufs=2))
        s3pool = ctx.enter_context(tc.tile_pool(name="s3", bufs=3))
        opool = ctx.enter_context(tc.tile_pool(name="og", bufs=2))

        # DMA issue order matters: the Sync HWDGE queue is FIFO and the
        # fabric is the bottleneck during ramp.  mmA_0 needs x0 half A + W,
        # so those go first; aux isn't needed until products_0 (~15us later).
        xt0 = xpool.tile([P, FT * KN], f16, tag="xc", name="x0")
        nc.sync.dma_start(
            out=_ap(xt0[:], 0, [[KN, FT], [1, KH * nt]]),
            in_=_rap(xr_b, 0, [[KN, P], [P * KN, FT], [1, KH * nt]]),
        )

        # resident W^T tiles: wsb[ft][p, g] , f = ft*128+p
        wsb = []
        for ft in range(FT):
            w_t = wpool.tile([P, F], f16, tag=f"w{ft}")
            nc.sync.dma_start(out=w_t[:], in_=wt[ft * P : (ft + 1) * P, :])
            wsb.append(w_t)

        # x0 half B
        nc.sync.dma_start(
            out=_ap(xt0[:], KH * nt, [[KN, FT], [1, KH * nt]]),
            in_=_rap(xr_b, KH * nt, [[KN, P], [P * KN, FT], [1, KH * nt]]),
        )

        # resident aux tile: [P, nch, FT, 2, nt] flattened
        auxt = wpool.tile([P, nch * FT * 2 * nt], f16, tag="aux")
        for c in range(nch):
            nc.sync.dma_start(
                out=_ap(auxt[:], c * FT * 2 * nt, [[2 * nt, FT], [1, 2 * nt]]),
                in_=_rap(
                    auxr_b,
                    c * F * 2 * nt,
                    [[2 * nt, P], [P * 2 * nt, FT], [1, 2 * nt]],
                ),
            )

        # Walrus only allows ONE sync wait per Matmult (waits ride the
        # LDWEIGHTS struct).  Warmup matmuls make PE observe each W-DMA
        # semaphore individually so later matmuls never wait on W.
        warm = papool.tile([P, KH, nt], f32, tag="pda")
        for ft in range(FT):
            nc.tensor.matmul(
                warm[:, 0, 0:1], wsb[ft][:, 0:P], wsb[ft][:, 0:1], start=True, stop=True
            )

        def emit_products(st):
            # p slots: 0..3 <- x(3,1,7,5)*d(1,3,5,7); 4,5 <- x(6,2)*d(2,6);
            #          6,7 <- aux(0,1)*d(0,4)
            xt, xo, dc = st["xt"], st["xo"], st["dc"]
            p = prpool.tile([P, K, nt], f16, tag="p")
            nc.vector.tensor_tensor(
                out=_ap(p[:], 0, [[1, 4 * nt]]),
                in0=_ap(xt, xo + 3 * nt, [[4 * nt, 2], [-2 * nt, 2], [1, nt]]),
                in1=_ap(dc[:], nt, [[4 * nt, 2], [2 * nt, 2], [1, nt]]),
                op=Alu.mult,
            )
            nc.vector.tensor_tensor(
                out=_ap(p[:], 4 * nt, [[1, 2 * nt]]),
                in0=_ap(xt, xo + 6 * nt, [[-4 * nt, 2], [1, nt]]),
                in1=_ap(dc[:], 2 * nt, [[4 * nt, 2], [1, nt]]),
                op=Alu.mult,
            )
            nc.vector.tensor_tensor(
                out=_ap(p[:], 6 * nt, [[1, 2 * nt]]),
                in0=st["ax"],
                in1=_ap(dc[:], 0, [[4 * nt, 2], [1, nt]]),
                op=Alu.mult,
            )
            st["p"] = p

        def emit_t1(st):
            p = st["p"]
            if T1_MODE == 1:
                nc.gpsimd.dma_start(
                    out=p[:, 4:8, :], in_=p[:, 0:4, :], accum_op=Alu.add
                )
            elif T1_MODE == 2:
                nc.gpsimd.dma_start(
                    out=p[:, 4:6, :], in_=p[:, 0:2, :], accum_op=Alu.add
                )
                nc.vector.tensor_tensor(
                    out=_ap(p[:], 6 * nt, [[1, 2 * nt]]),
                    in0=_ap(p[:], 2 * nt, [[1, 2 * nt]]),
                    in1=_ap(p[:], 6 * nt, [[1, 2 * nt]]),
                    op=Alu.add,
                )
            else:
                nc.vector.tensor_tensor(
                    out=_ap(p[:], 4 * nt, [[1, 4 * nt]]),
                    in0=_ap(p[:], 0, [[1, 4 * nt]]),
                    in1=_ap(p[:], 4 * nt, [[1, 4 * nt]]),
                    op=Alu.add,
                )

        def emit_t2_kf(st):
            p = st["p"]
            t2 = s2pool.tile([P, 2, nt], f16, tag="t2")
            nc.vector.tensor_tensor(
                out=_ap(t2[:], 0, [[1, 2 * nt]]),
                in0=_ap(p[:], 4 * nt, [[1, 2 * nt]]),
                in1=_ap(p[:], 6 * nt, [[1, 2 * nt]]),
                op=Alu.add,
            )
            kf = s3pool.tile([P, nt], f16, tag="kf")
            nc.vector.tensor_tensor(
                out=kf[:], in0=t2[:, 0, :], in1=t2[:, 1, :], op=Alu.add
            )
            st["kf"] = kf

        def emit_gate(st):
            gate = s3pool.tile([P, nt], f16, tag="gate")
            nc.scalar.activation(
                out=gate[:], in_=st["kf"][:], func=ActF.Relu, scale=6.0
            )
            st["gate"] = gate

        def emit_og_o2(st):
            dc, xt, xo = st["dc"], st["xt"], st["xo"]
            og = opool.tile([P, K, nt], f16, tag="og", bufs=1)
            nc.vector.tensor_tensor(
                out=_ap(og[:], 0, [[1, K * nt]]),
                in0=_ap(dc[:], 0, [[1, K * nt]]),
                in1=_ap(st["gate"][:], 0, [[0, K], [1, nt]]),
                op=Alu.mult,
            )
            o2 = opool.tile([P, K, nt], f16, tag="o2")
            nc.vector.tensor_tensor(
                out=_ap(o2[:], 0, [[1, K * nt]]),
                in0=_ap(og[:], 0, [[1, K * nt]]),
                in1=_ap(xt, xo, [[1, K * nt]]),
                op=Alu.add,
            )
            st["o2"] = o2

        def emit_out(st):
            c, gt = st["c"], st["gt"]
            nc.scalar.dma_start(
                out=_rap(
                    outr_b,
                    (c * F + gt * P) * KN,
                    [[KN, P], [1, KN]],
                ),
                in_=st["o2"][:],
            )

        prev = None   # iter m-1: products/t1/t2/kf/gate pending
        prev2 = None  # iter m-2: og/o2/out pending

        for c in range(nch):
            # one x tile per chunk: [P, FT*K*nt] (f = ft*128 + p)
            if c == 0:
                xt = xt0
            else:
                xt = xpool.tile([P, FT * KN], f16, tag="xc")
                nc.sync.dma_start(
                    out=xt[:],
                    in_=_rap(
                        xr_b,
                        c * F * KN,
                        [[KN, P], [P * KN, FT], [1, KN]],
                    ),
                )
            for gt in range(FT):
                xo = gt * KN
                dc = dcpool.tile([P, K, nt], f16, tag="dc")

                # ScalarE head: dcB of the PREVIOUS iter
                if prev is not None:
                    nc.scalar.copy(out=prev["dc"][:, KH:K, :], in_=prev["pdB"][:])
                # ---- matmul halves -> PSUM ----
                pds = []
                for half, pool in ((0, papool), (1, pbpool)):
                    pd = pool.tile([P, KH, nt], f32, tag=("pda", "pdb")[half])
                    # Dummy matmul absorbs the PSUM-slot-release wait
                    # (1-wait limit on Matmult structs).
                    nc.tensor.matmul(
                        pd[:, 0, 0:1], wsb[0][:, 0:P], wsb[0][:, 0:1],
                        start=True, stop=True,
                    )
                    k0 = half * KH
                    for ft in range(FT):
                        for jj in range(KH):
                            nc.tensor.matmul(
                                pd[:, jj, :],
                                wsb[ft][:, gt * P : (gt + 1) * P],
                                _ap(xt[:], ft * KN + (k0 + jj) * nt, [[1, nt]]),
                                start=(ft == 0),
                                stop=(ft == FT - 1),
                            )
                    pds.append(pd)
                    if half == 0:
                        nc.scalar.copy(out=dc[:, 0:KH, :], in_=pd[:])

                # ---- elementwise streams ----
                if prev is not None:
                    emit_products(prev)
                    emit_t1(prev)
                if prev2 is not None:
                    emit_og_o2(prev2)
                if prev is not None:
                    emit_t2_kf(prev)
                    emit_gate(prev)
                if prev2 is not None:
                    emit_out(prev2)

                prev2 = prev
                prev = {
                    "dc": dc, "xt": xt[:], "xo": xo,
                    "ax": _ap(auxt[:], (c * FT + gt) * 2 * nt, [[1, 2 * nt]]),
                    "pdB": pds[1], "c": c, "gt": gt,
                }

        # ---- drain ----
        nc.scalar.copy(out=prev["dc"][:, KH:K, :], in_=prev["pdB"][:])
        emit_products(prev)
        emit_t1(prev)
        emit_og_o2(prev2)
        emit_t2_kf(prev)
        emit_gate(prev)
        emit_out(prev2)
        emit_og_o2(prev)
        emit_out(prev)

    _split_waits(nc)
    return nc


# Engine datapath structs (Matmult/TT/STT/Act/...) only carry ONE sync wait on
# TRN2 walrus; sequencer instructions (NoOp) can each carry one more.  Hoist
# surplus waits onto same-engine NoOps placed just before the instruction.
def _split_waits(nc):
    nnop = 0
    for fn in nc.m.functions:
        for blk in fn.blocks:
            out = []
            for inst in blk.instructions:
                si = inst.sync_info
                if si is not None and si.on_wait and len(si.on_wait) > 1:
                    for w in si.on_wait[:-1]:
                        nop = mybir.InstNoOp(
                            name=f"{inst.name}-sw{nnop}",
                            opcode="NoOp",
                            engine=inst.engine,
                            sync_info=mybir.SyncInfo(on_wait=[w], on_update=[]),
                        )
                        nnop += 1
                        out.append(nop)
                    inst.sync_info = mybir.SyncInfo(
                        on_wait=[si.on_wait[-1]], on_update=list(si.on_update)
                    )
                out.append(inst)
            blk.instructions[:] = out
    return nc


_NC_CACHE = {}

NCH, NT = N // 512, 512


def _get_nc(n_total=N, nt=NT):
    key = (n_total, nt)
    if key not in _NC_CACHE:
        _NC_CACHE[key] = build_nc(n_total, nt)
    return _NC_CACHE[key]


def _to_f16(a: np.ndarray) -> np.ndarray:
    return np.ascontiguousarray(a.astype(np.float16))


def _chunked(a: np.ndarray) -> np.ndarray:
    # [F, C, N] -> chunk-contiguous [nch, F, C, nt]
    Fd, Cd, Nd = a.shape
    return np.ascontiguousarray(
        a.reshape(Fd, Cd, NCH, NT).transpose(2, 0, 1, 3)
    )


def make_in_maps(x: np.ndarray, W: np.ndarray):
    wt = _to_f16(W.T.copy())
    x16 = _to_f16(x)
    xa = np.stack(
        [2.0 * x[:, :, 0, :] - x[:, :, 4, :], 2.0 * x[:, :, 4, :] - x[:, :, 0, :]],
        axis=2,
    )
    xa16 = _to_f16(xa)
    return [
        {"xr": _chunked(x16[b]), "wt": wt, "auxr": _chunked(xa16[b])}
        for b in range(B)
    ]


def post(res) -> np.ndarray:
    # gather + un-chunk: [nch, F, K, nt] -> [F, K, N]
    return np.stack(
        [
            res.results[b]["outr"]
            .transpose(1, 2, 0, 3)
            .reshape(F, K, N)
            .astype(np.float32)
            for b in range(B)
        ],
        axis=0,
    )


def kernel(x: np.ndarray, W: np.ndarray) -> np.ndarray:
    assert x.shape == (B, F, K, N) and W.shape == (F, F)
    in_maps = make_in_maps(x, W)
    nc = _get_nc()
    res = run_bass_kernel_spmd(nc, in_maps, list(range(B)))
    return post(res)


if __name__ == "__main__":
    xs = np.random.randn(B, F, K, N).astype(np.float32)
    Ws = (np.random.randn(F, F) / np.sqrt(F)).astype(np.float32)
    o = kernel(xs, Ws)
    print(o.shape, o.dtype)
